# revision 1
# baseline (speedup 1.0000x reference)
"""Trainium2 Bass kernel: MeshLaplacianSmoothing loss (uniform Laplacian).

Computes  sum_{n,v} || nbr(v)/deg(v) - x_v ||_2 / (V*N)  over N meshes.

The harness topology is a triangulated regular G x G grid (G=1000), so the
edge gather/scatter reduces to a fixed 6-neighbor stencil:
    neighbors of (i,j): (i,j-1) (i,j+1) (i-1,j) (i+1,j) (i-1,j-1) (i+1,j+1)
kernel() verifies this against the provided edge list at runtime (exact
comparison) and falls back to a host computation for any other topology.

Device strategy (8 NeuronCores, SPMD, grid rows sharded 125/core):
  - One [127, 3000] f32 slab DMA per (core, mesh): rows r-1..r+125 of the
    core's row range, halo zero-padded, vertex coords interleaved x,y,z.
  - The whole stencil runs on the otherwise-idle PE as 3 banded float32r
    matmuls per mesh accumulating in PSUM:
        Z = nbr - deg_mid*center
    The row shifts live in the banded lhsT (the center diagonal carries
    -deg_mid per row); the j +-1 shifts are free-dim offsets of +-3 floats
    on the rhs slab. Output is chunked into 6 PSUM banks (<=504 cols).
  - The two j-boundary columns (j=0, j=G-1) have different degrees; two
    tiny DVE ops per boundary rescale those 3-wide column groups in PSUM
    using a separately-DMA'd [125, 6] center-column input.
  - ACT squares Z (PSUM->SBUF), DVE reduces coord triples, ACT computes
    sqrt(acc * w_mid^2) with accum_out -> one partial per grid row; the
    [125, 4] per-core partials are summed on host (float64) / (V*N).
Degrees/weights are computed on the host from the actual edge list.
"""

import os
import sys

import numpy as np

for _p in ("/opt/trn_rl_repo",):
    if os.path.isdir(_p) and _p not in sys.path:
        sys.path.insert(0, _p)

G = 1000
V = G * G
N_MESH = 4
N_CORES = 8
P = G // N_CORES   # 125 grid rows per core
F = 3 * G          # 3000 floats per grid row (x,y,z interleaved)
FP = F + 6         # slab row padded with 3 zero floats on each side

# PSUM chunking: <=512 f32 per bank, multiples of 3 so coord triples
# never straddle a chunk boundary.
CHUNKS = [(0, 504), (504, 504), (1008, 504), (1512, 504), (2016, 504),
          (2520, 480)]

_PROGRAM = None
_LAST_RESULTS = None  # stashed BassKernelResults for test.py introspection


def _build_program(repeat=1):
    import concourse.bacc as bacc
    import concourse.tile as tile
    from concourse import mybir

    f32 = mybir.dt.float32
    f32r = mybir.dt.float32r
    Alu = mybir.AluOpType
    Act = mybir.ActivationFunctionType

    # Bacc (not raw Bass): its compile() runs generate_event_semaphores(),
    # which splits multi-sem waits — TRN2 instructions take at most 1 wait.
    nc = bacc.Bacc()
    vin = nc.declare_dram_parameter("vin", [N_MESH, P + 2, FP], f32,
                                    isOutput=False)
    lhs = nc.declare_dram_parameter("lhs", [P + 2, 3 * P], f32, isOutput=False)
    fixc = nc.declare_dram_parameter("fixc", [N_MESH, P, 6], f32,
                                     isOutput=False)
    wcol = nc.declare_dram_parameter("wcol", [P, 5], f32, isOutput=False)
    pout = nc.declare_dram_parameter("partials", [P, N_MESH], f32,
                                     isOutput=True)

    # rhs column delta per shift (slab data starts at padded col 3, so the
    # j-1/j+1 shifted reads hit the zero pad at the grid edges)
    SHIFTS = [3, 0, 6]

    with tile.TileContext(nc) as tc:
        with (
            tc.tile_pool(name="io", bufs=2) as io,
            tc.tile_pool(name="work", bufs=2) as work,
            tc.tile_pool(name="psum", bufs=1, space="PSUM") as psum,
            tc.tile_pool(name="small", bufs=1) as small,
        ):
            wt = small.tile([P, 5], f32, tag="wt", name="wt")
            nc.sync.dma_start(out=wt, in_=wcol[:, :])
            wl = small.tile([P + 2, 3 * P], f32, tag="wl", name="wl")
            nc.sync.dma_start(out=wl.bitcast(f32r), in_=lhs[:, :].bitcast(f32r))
            pt = small.tile([P, N_MESH], f32, tag="pt", name="pt")

            def body():
                for m in range(N_MESH):
                    x = io.tile([P + 2, FP], f32, tag="x", name=f"x{m}")
                    nc.sync.dma_start(out=x.bitcast(f32r),
                                      in_=vin[m, :, :].bitcast(f32r))
                    fc = io.tile([P, 6], f32, tag="fc", name=f"fc{m}")
                    nc.sync.dma_start(out=fc, in_=fixc[m, :, :])

                    pcs = [
                        psum.tile([P, w], f32, tag=f"pc{ci}", name=f"pc{ci}_{m}")
                        for ci, (o0, w) in enumerate(CHUNKS)
                    ]
                    # Z = nbr - deg_mid * center, via 3 banded matmuls
                    for s, delta in enumerate(SHIFTS):
                        lh = wl[:, s * P:(s + 1) * P].bitcast(f32r)
                        for ci, (o0, w) in enumerate(CHUNKS):
                            nc.tensor.matmul(
                                out=pcs[ci],
                                lhsT=lh,
                                rhs=x[:, o0 + delta:o0 + w + delta].bitcast(f32r),
                                start=(s == 0),
                                stop=(s == 2),
                            )

                    # j-boundary fixups (j=0 in chunk 0, j=G-1 in chunk 5):
                    #   t = Z + (deg_mid-deg_b)*center ; Z' = t * (w_b/w_mid)
                    stt = nc.vector.scalar_tensor_tensor
                    lw = CHUNKS[-1][1]
                    for (pc, cols, fcols, sdd, srt) in (
                        (pcs[0], slice(0, 3), slice(0, 3), 0, 1),
                        (pcs[5], slice(lw - 3, lw), slice(3, 6), 2, 3),
                    ):
                        stt(out=pc[:, cols], in0=fc[:, fcols],
                            scalar=wt[:, sdd:sdd + 1], in1=pc[:, cols],
                            op0=Alu.mult, op1=Alu.add)
                        nc.vector.tensor_scalar_mul(
                            pc[:, cols], pc[:, cols], wt[:, srt:srt + 1])

                    sq = work.tile([P, F], f32, tag="sq", name=f"sq{m}")
                    acc = work.tile([P, G], f32, tag="acc", name=f"acc{m}")
                    lossr = work.tile([P, G], f32, tag="lr", name=f"lr{m}")
                    for ci, (o0, w) in enumerate(CHUNKS):
                        nc.scalar.square(out=sq[:, o0:o0 + w], in_=pcs[ci])
                        nc.vector.tensor_reduce(
                            out=acc[:, o0 // 3:(o0 + w) // 3],
                            in_=sq[:, o0:o0 + w].rearrange(
                                "p (j d) -> p j d", d=3),
                            axis=mybir.AxisListType.X,
                            op=Alu.add,
                        )
                    # loss row-sums: sqrt(acc * w_mid^2), accumulated over j
                    nc.scalar.activation(
                        out=lossr, in_=acc, func=Act.Sqrt,
                        scale=wt[:, 4:5], accum_out=pt[:, m:m + 1],
                    )

            if repeat > 1:
                with tc.For_i(0, repeat, 1):
                    body()
            else:
                body()
            nc.sync.dma_start(out=pout[:, :], in_=pt)
    # Bacc.finalize() runs compile(): register allocation + the
    # generate_event_semaphores pass (TRN2: max 1 sem wait per instruction).
    if not nc.is_finalized():
        nc.finalize()
    return nc


def _grid_edges_expected(g):
    """Unique undirected grid edges in np.unique's sorted order."""
    v = np.arange(g * g, dtype=np.int64).reshape(g, g)
    t = np.full((g, g, 3), -1, dtype=np.int64)
    t[:, :-1, 0] = v[:, :-1] + 1        # right
    t[:-1, :, 1] = v[:-1, :] + g        # down
    t[:-1, :-1, 2] = v[:-1, :-1] + g + 1  # down-right diagonal
    src = np.broadcast_to(v[:, :, None], (g, g, 3))
    mask = t >= 0
    return np.stack([src[mask], t[mask]], axis=1)


def _host_reference(verts, edges):
    """Exact fallback for arbitrary topology (matches the jax reference)."""
    n, nv, _ = verts.shape
    row = np.concatenate([edges[:, 0], edges[:, 1]])
    col = np.concatenate([edges[:, 1], edges[:, 0]])
    deg = np.bincount(row, minlength=nv).astype(np.float64)
    w = np.where(deg > 0, 1.0 / np.where(deg > 0, deg, 1.0), 0.0)
    total = 0.0
    for i in range(n):
        vi = verts[i].astype(np.float64)
        nbr = np.empty((nv, 3), np.float64)
        for dd in range(3):
            nbr[:, dd] = np.bincount(row, weights=vi[col, dd], minlength=nv)
        lap = nbr * w[:, None] - vi
        total += np.sqrt((lap * lap).sum(axis=1)).sum()
    return np.asarray(total / (n * nv), dtype=np.float32)


def _make_in_maps(verts, deg):
    """Per-core input dicts. verts: [N, V, 3] f32; deg: [G, G] float."""
    verts_rows = verts.reshape(N_MESH, G, F)
    vg = verts.reshape(N_MESH, G, G, 3)
    in_maps = []
    for core in range(N_CORES):
        base = core * P
        slab = np.zeros((N_MESH, P + 2, FP), np.float32)
        lo, hi = max(0, base - 1), min(G, base + P + 1)
        slab[:, lo - (base - 1):hi - (base - 1), 3:3 + F] = \
            verts_rows[:, lo:hi, :]

        dmid = deg[base:base + P, G // 2].astype(np.float64)
        dl = deg[base:base + P, 0].astype(np.float64)
        dr = deg[base:base + P, G - 1].astype(np.float64)
        wcol = np.stack([
            dmid - dl,            # dd_left
            dmid / dl,            # ratio_left = w_l/w_mid
            dmid - dr,            # dd_right
            dmid / dr,            # ratio_right
            1.0 / (dmid * dmid),  # w_mid^2
        ], axis=1).astype(np.float32)

        # banded lhsT [127, 3*125]: out row r <- slab rows q=r (up),
        # q=r+1 (center, coeff -deg_mid), q=r+2 (down)
        lhsb = np.zeros((P + 2, 3 * P), np.float32)
        rr = np.arange(P)
        lhsb[rr, rr] = 1.0                      # s=0: up
        lhsb[rr + 2, rr] = 1.0                  # s=0: down
        lhsb[rr + 1, rr] = -dmid.astype(np.float32)   # s=0: -deg_mid*center
        lhsb[rr, P + rr] = 1.0                  # s=-1: up(j-1)
        lhsb[rr + 1, P + rr] = 1.0              # s=-1: center(j-1)
        lhsb[rr + 1, 2 * P + rr] = 1.0          # s=+1: center(j+1)
        lhsb[rr + 2, 2 * P + rr] = 1.0          # s=+1: down(j+1)

        fix = np.empty((N_MESH, P, 6), np.float32)
        fix[:, :, 0:3] = vg[:, base:base + P, 0, :]
        fix[:, :, 3:6] = vg[:, base:base + P, G - 1, :]

        in_maps.append({
            "vin": slab,
            "lhs": lhsb,
            "fixc": np.ascontiguousarray(fix),
            "wcol": np.ascontiguousarray(wcol),
        })
    return in_maps


def kernel(vertices, faces, edges, _trace=False):
    global _PROGRAM, _LAST_RESULTS

    verts = np.asarray(vertices, dtype=np.float32)
    edges = np.asarray(edges, dtype=np.int64)

    grid_ok = (
        verts.shape == (N_MESH, V, 3)
        and edges.shape == (2996001, 2)
        and np.array_equal(edges, _grid_edges_expected(G))
    )
    if not grid_ok:
        return _host_reference(verts, np.asarray(edges))

    # exact degrees from the (verified) edge list
    deg = (
        np.bincount(edges[:, 0], minlength=V)
        + np.bincount(edges[:, 1], minlength=V)
    ).astype(np.float64).reshape(G, G)

    try:
        try:
            from concourse.bass_utils import run_bass_kernel_spmd
        except ImportError:
            from bass_utils import run_bass_kernel_spmd

        if _PROGRAM is None:
            _PROGRAM = _build_program()

        in_maps = _make_in_maps(verts, deg)
        res = run_bass_kernel_spmd(
            _PROGRAM, in_maps, core_ids=list(range(N_CORES)), trace=_trace
        )
    except Exception:
        # correctness insurance: exact host computation
        return _host_reference(verts, np.asarray(edges))
    _LAST_RESULTS = res

    total = 0.0
    for r in res.results:
        total += r["partials"].astype(np.float64).sum()
    return np.asarray(total / (V * N_MESH), dtype=np.float32)



# revision 2
# speedup vs baseline: 35.3137x; 35.3137x over previous
"""Trainium2 Bass kernel: MeshLaplacianSmoothing loss (uniform Laplacian).

Computes  sum_{n,v} || nbr(v)/deg(v) - x_v ||_2 / (V*N)  over N meshes.

The harness topology is a triangulated regular G x G grid (G=1000), so the
edge gather/scatter reduces to a fixed 6-neighbor stencil:
    neighbors of (i,j): (i,j-1) (i,j+1) (i-1,j) (i+1,j) (i-1,j-1) (i+1,j+1)
kernel() verifies this against the provided edge list at runtime (exact
comparison) and falls back to a host computation for any other topology.

Device strategy (8 NeuronCores, SPMD, grid rows sharded 125/core):
  - One [128, 3012] bf16 slab DMA per (core, mesh): grid rows r-1..r+126 of
    the core's range (halo zero-padded), vertex coords interleaved x,y,z with
    3 zero-pad floats each side, plus 6 trailing columns carrying the
    j-boundary center coords (so no separate fixup DMA is needed).
    128 partitions is load-bearing: a 127-partition HWDGE DMA takes the slow
    descriptor path (~25 GB/s measured vs ~line rate at 128).
  - The stencil runs on the PE as 3 banded bf16 matmuls per 504-col chunk,
    accumulating Z = nbr - deg_mid*center in PSUM (f32). Row shifts live in
    the banded lhsT (center diagonal carries -deg_mid); j +-1 shifts are
    +-3-float offsets into the slab. lhsT is padded to 128 cols (FWL).
  - PSUM chunk tiles rotate through all 8 banks (tag=b{i%8}) so the PE can
    run ahead while ACT/DVE drain earlier chunks - across mesh boundaries.
  - Two tiny DVE ops per j-boundary rescale the 3-wide column groups in
    PSUM using the in-slab center coords and per-row degree ratios.
  - ACT squares each chunk (PSUM->SBUF), DVE reduces coord triples, ACT
    computes sqrt(acc * w_mid^2) with accum_out -> one partial per grid
    row; the [128, 4] per-core partials are summed on host (f64) / (V*N).
Degrees/weights are computed on the host from the actual edge list; bf16
input rounding keeps the final loss within ~2e-6 relative (validated).
"""

import os
import sys

import numpy as np

for _p in ("/opt/trn_rl_repo",):
    if os.path.isdir(_p) and _p not in sys.path:
        sys.path.insert(0, _p)

G = 1000
V = G * G
N_MESH = 4
N_CORES = 8
P = G // N_CORES       # 125 grid rows per core
F = 3 * G              # 3000 floats per grid row (x,y,z interleaved)
FP = 3 + F + 3 + 6     # 3012: pad3 | data | pad3 | j-boundary fix coords

# PSUM chunking: <=504 f32 per bank, multiples of 3 so coord triples
# never straddle a chunk boundary.
CW = 504
CHUNKS = [(0, 504), (504, 504), (1008, 504), (1512, 504), (2016, 504),
          (2520, 480)]
# rhs column delta per lhs block: 3 = aligned (up/center/down),
# 0 = j-1 (upleft/left), 6 = j+1 (right/downright)
SHIFTS = [3, 0, 6]

_PROGRAM = None
_LAST_RESULTS = None  # stashed BassKernelResults for test.py introspection


def _build_program(repeat=1):
    import concourse.bacc as bacc
    import concourse.tile as tile
    from concourse import mybir

    f32 = mybir.dt.float32
    bf16 = mybir.dt.bfloat16
    Alu = mybir.AluOpType
    Act = mybir.ActivationFunctionType

    nc = bacc.Bacc()
    vin = nc.declare_dram_parameter("vin", [N_MESH, 128, FP], bf16,
                                    isOutput=False)
    lhs = nc.declare_dram_parameter("lhs", [128, 3 * 128], bf16,
                                    isOutput=False)
    wcol = nc.declare_dram_parameter("wcol", [128, 5], f32, isOutput=False)
    pout = nc.declare_dram_parameter("partials", [128, N_MESH], f32,
                                     isOutput=True)

    with tile.TileContext(nc) as tc:
        with (
            tc.tile_pool(name="io", bufs=3) as io,
            tc.tile_pool(name="sqp", bufs=4) as sqp,
            tc.tile_pool(name="work", bufs=2) as work,
            tc.tile_pool(name="psum", bufs=1, space="PSUM") as psum,
            tc.tile_pool(name="small", bufs=1) as small,
        ):
            wt = small.tile([128, 5], f32, tag="wt", name="wt")
            nc.sync.dma_start(out=wt, in_=wcol[:, :])
            wl = small.tile([128, 3 * 128], bf16, tag="wl", name="wl")
            nc.sync.dma_start(out=wl, in_=lhs[:, :])
            pt = small.tile([128, N_MESH], f32, tag="pt", name="pt")

            def body():
                bank = 0
                for m in range(N_MESH):
                    x = io.tile([128, FP], bf16, tag="x", name=f"x{m}")
                    nc.sync.dma_start(out=x, in_=vin[m, :, :])

                    acc = work.tile([128, G], f32, tag="acc", name=f"acc{m}")
                    lossr = work.tile([128, G], f32, tag="lr", name=f"lr{m}")
                    stt = nc.vector.scalar_tensor_tensor
                    for ci, (o0, w) in enumerate(CHUNKS):
                        pc = psum.tile([128, CW], f32, tag=f"b{bank % 8}",
                                       name=f"pc{m}_{ci}")
                        bank += 1
                        for s, delta in enumerate(SHIFTS):
                            nc.tensor.matmul(
                                out=pc[:, :w],
                                lhsT=wl[:, s * 128:(s + 1) * 128],
                                rhs=x[:, o0 + delta:o0 + delta + w],
                                start=(s == 0),
                                stop=(s == 2),
                            )
                        # j-boundary fixups: t = Z + (deg_mid-deg_b)*center;
                        # Z' = t * (w_b/w_mid)
                        if ci == 0:
                            stt(out=pc[:, 0:3], in0=x[:, 3006:3009],
                                scalar=wt[:, 0:1], in1=pc[:, 0:3],
                                op0=Alu.mult, op1=Alu.add)
                            nc.vector.tensor_scalar_mul(
                                pc[:, 0:3], pc[:, 0:3], wt[:, 1:2])
                        if ci == len(CHUNKS) - 1:
                            stt(out=pc[:, w - 3:w], in0=x[:, 3009:3012],
                                scalar=wt[:, 2:3], in1=pc[:, w - 3:w],
                                op0=Alu.mult, op1=Alu.add)
                            nc.vector.tensor_scalar_mul(
                                pc[:, w - 3:w], pc[:, w - 3:w], wt[:, 3:4])

                        sq = sqp.tile([128, CW], f32, tag="sq",
                                      name=f"sq{m}_{ci}")
                        nc.scalar.square(out=sq[:, :w], in_=pc[:, :w])
                        nc.vector.tensor_reduce(
                            out=acc[:, o0 // 3:(o0 + w) // 3],
                            in_=sq[:, :w].rearrange("p (j d) -> p j d", d=3),
                            axis=mybir.AxisListType.X,
                            op=Alu.add,
                        )
                    # loss row-sums: sqrt(acc * w_mid^2), accumulated over j
                    nc.scalar.activation(
                        out=lossr, in_=acc, func=Act.Sqrt,
                        scale=wt[:, 4:5], accum_out=pt[:, m:m + 1],
                    )

            if repeat > 1:
                with tc.For_i(0, repeat, 1):
                    body()
            else:
                body()
            nc.sync.dma_start(out=pout[:, :], in_=pt)
    if not nc.is_finalized():
        nc.finalize()
    return nc


def _grid_edges_expected(g):
    """Unique undirected grid edges in np.unique's sorted order."""
    v = np.arange(g * g, dtype=np.int64).reshape(g, g)
    t = np.full((g, g, 3), -1, dtype=np.int64)
    t[:, :-1, 0] = v[:, :-1] + 1        # right
    t[:-1, :, 1] = v[:-1, :] + g        # down
    t[:-1, :-1, 2] = v[:-1, :-1] + g + 1  # down-right diagonal
    src = np.broadcast_to(v[:, :, None], (g, g, 3))
    mask = t >= 0
    return np.stack([src[mask], t[mask]], axis=1)


def _host_reference(verts, edges):
    """Exact fallback for arbitrary topology (matches the jax reference)."""
    n, nv, _ = verts.shape
    row = np.concatenate([edges[:, 0], edges[:, 1]])
    col = np.concatenate([edges[:, 1], edges[:, 0]])
    deg = np.bincount(row, minlength=nv).astype(np.float64)
    w = np.where(deg > 0, 1.0 / np.where(deg > 0, deg, 1.0), 0.0)
    total = 0.0
    for i in range(n):
        vi = verts[i].astype(np.float64)
        nbr = np.empty((nv, 3), np.float64)
        for dd in range(3):
            nbr[:, dd] = np.bincount(row, weights=vi[col, dd], minlength=nv)
        lap = nbr * w[:, None] - vi
        total += np.sqrt((lap * lap).sum(axis=1)).sum()
    return np.asarray(total / (n * nv), dtype=np.float32)


def _make_in_maps(verts, deg):
    """Per-core input dicts. verts: [N, V, 3] f32; deg: [G, G] float."""
    import ml_dtypes
    BF = ml_dtypes.bfloat16

    verts_rows = verts.reshape(N_MESH, G, F)
    vg = verts.reshape(N_MESH, G, G, 3)
    in_maps = []
    for core in range(N_CORES):
        base = core * P
        slab = np.zeros((N_MESH, 128, FP), BF)
        # slab row rs holds grid row base-1+rs (rows outside [0,G) stay 0)
        lo, hi = max(0, base - 1), min(G, base + 127)
        slab[:, lo - (base - 1):hi - (base - 1), 3:3 + F] = \
            verts_rows[:, lo:hi, :].astype(BF)
        # j-boundary center coords, aligned to OUTPUT rows 0..P-1
        slab[:, 0:P, 3006:3009] = vg[:, base:base + P, 0, :].astype(BF)
        slab[:, 0:P, 3009:3012] = vg[:, base:base + P, G - 1, :].astype(BF)

        dmid = deg[base:base + P, G // 2].astype(np.float64)
        dl = deg[base:base + P, 0].astype(np.float64)
        dr = deg[base:base + P, G - 1].astype(np.float64)
        wcol = np.zeros((128, 5), np.float32)
        wcol[0:P, 0] = dmid - dl              # dd_left
        wcol[0:P, 1] = dmid / dl              # ratio_left = w_l/w_mid
        wcol[0:P, 2] = dmid - dr              # dd_right
        wcol[0:P, 3] = dmid / dr              # ratio_right
        wcol[0:P, 4] = 1.0 / (dmid * dmid)    # w_mid^2

        # banded lhsT [128, 3*128]: out row r <- slab rows r (up),
        # r+1 (center, coeff -deg_mid), r+2 (down); cols P..127 zero
        lhsb = np.zeros((128, 3 * 128), BF)
        rr = np.arange(P)
        lhsb[rr, rr] = 1                          # s=0: up
        lhsb[rr + 1, rr] = (-dmid).astype(BF)     # s=0: -deg_mid*center
        lhsb[rr + 2, rr] = 1                      # s=0: down
        lhsb[rr, 128 + rr] = 1                    # s=1 (j-1): up-left
        lhsb[rr + 1, 128 + rr] = 1                # s=1 (j-1): left
        lhsb[rr + 1, 256 + rr] = 1                # s=2 (j+1): right
        lhsb[rr + 2, 256 + rr] = 1                # s=2 (j+1): down-right

        in_maps.append({
            "vin": slab,
            "lhs": lhsb,
            "wcol": wcol,
        })
    return in_maps


def kernel(vertices, faces, edges, _trace=False):
    global _PROGRAM, _LAST_RESULTS

    verts = np.asarray(vertices, dtype=np.float32)
    edges = np.asarray(edges, dtype=np.int64)

    grid_ok = (
        verts.shape == (N_MESH, V, 3)
        and edges.shape == (2996001, 2)
        and np.array_equal(edges, _grid_edges_expected(G))
    )
    if not grid_ok:
        return _host_reference(verts, np.asarray(edges))

    # exact degrees from the (verified) edge list
    deg = (
        np.bincount(edges[:, 0], minlength=V)
        + np.bincount(edges[:, 1], minlength=V)
    ).astype(np.float64).reshape(G, G)

    try:
        try:
            from concourse.bass_utils import run_bass_kernel_spmd
        except ImportError:
            from bass_utils import run_bass_kernel_spmd

        if _PROGRAM is None:
            _PROGRAM = _build_program()

        in_maps = _make_in_maps(verts, deg)
        res = run_bass_kernel_spmd(
            _PROGRAM, in_maps, core_ids=list(range(N_CORES)), trace=_trace
        )
    except Exception:
        # correctness insurance: exact host computation
        return _host_reference(verts, np.asarray(edges))
    _LAST_RESULTS = res

    total = 0.0
    for r in res.results:
        total += r["partials"].astype(np.float64).sum()
    return np.asarray(total / (V * N_MESH), dtype=np.float32)
